# revision 9
# baseline (speedup 1.0000x reference)
"""Trainium2 Bass kernel for nn_AttnProcessor (DIFT nearest-neighbor sparse attention).

8-core SPMD: attention heads sharded across cores (1 head/core, all 4 batches);
the DIFT NN map is computed with ref-columns sharded (128 ref tokens/core) and
combined with a tiny AllGather; the output projection is token-sharded after an
AllGather of per-head attention outputs.
"""
import os
import sys

for _p in ("/root/.axon_site/_ro/trn_rl_repo", "/opt/trn_rl_repo"):
    if os.path.isdir(_p) and _p not in sys.path:
        sys.path.append(_p)

import numpy as np

import concourse.bass as bass
import concourse.mybir as mybir
import concourse.tile as tile
from concourse import bacc
from concourse import bass_utils
from concourse.bass import ts, ds
from concourse.masks import make_identity

FP = mybir.dt.float32
U32 = mybir.dt.uint32
AF = mybir.ActivationFunctionType
OP = mybir.AluOpType

NCORES = 8
B, S, C, H, CD = 4, 1024, 640, 8, 1280
D = C // H              # 80 head dim
SUMROW = 96             # ones column lands on a valid partition base
DA = SUMROW + 1         # v augmented: cols [80,96) zero, col 96 = ones
TOK = B * S             # 4096
SH = S // NCORES        # 128 ref/tgt rows per core
P = 128
GEN, REF = 2, 3
SCALE = float(1.0 / np.sqrt(np.float32(D)))
NEG = -1e9
THRESH = 0.7
KCH = C // P            # 5 contraction chunks over C
CDCH = CD // P          # 10 contraction chunks over CD
NT = S // P             # 8 token tiles per batch
NSL = TOK // NCORES     # 512 output tokens per core

LAST_RESULTS = None


def build_program(debug_outputs=False, probe=False):
    nc = bacc.Bacc("TRN2", target_bir_lowering=False, debug=False, num_devices=NCORES)

    x_T = nc.dram_tensor("x_T", [C, TOK], FP, kind="ExternalInput")
    tnT = nc.dram_tensor("tnT", [CD, S], FP, kind="ExternalInput")
    refsh = nc.dram_tensor("refsh", [SH, CD], FP, kind="ExternalInput")
    tgtsh = nc.dram_tensor("tgtsh", [SH, CD], FP, kind="ExternalInput")
    maskv = nc.dram_tensor("maskv", [1, SH], FP, kind="ExternalInput")
    ibase = nc.dram_tensor("ibase", [P, 1], FP, kind="ExternalInput")
    wq_d = nc.dram_tensor("wq", [C, D], FP, kind="ExternalInput")
    wk_d = nc.dram_tensor("wk", [C, D], FP, kind="ExternalInput")
    wv_d = nc.dram_tensor("wv", [C, D], FP, kind="ExternalInput")
    woT_d = nc.dram_tensor("woT", [C, C], FP, kind="ExternalInput")
    boc_d = nc.dram_tensor("boc", [P, KCH], FP, kind="ExternalInput")
    yidx_d = nc.dram_tensor("yidx", [P, KCH], U32, kind="ExternalInput")
    y_out = nc.dram_tensor("y_out", [C, NSL], FP, kind="ExternalOutput")
    if debug_outputs:
        dbg_idx = nc.dram_tensor("dbg_idx", [P, NT], U32, kind="ExternalOutput")
        dbg_dist = nc.dram_tensor("dbg_dist", [P, NT], FP, kind="ExternalOutput")
    if probe:
        FR = mybir.dt.float32r
        pa_d = nc.dram_tensor("pa", [P, P], FP, kind="ExternalInput")
        pb_d = nc.dram_tensor("pb", [P, 512], FP, kind="ExternalInput")
        par_d = nc.dram_tensor("par", [P, P], FR, kind="ExternalInput")
        pbr_d = nc.dram_tensor("pbr", [P, 512], FR, kind="ExternalInput")
        probe32 = nc.dram_tensor("probe32", [P, 512], FP, kind="ExternalOutput")
        prober = nc.dram_tensor("prober", [P, 512], FP, kind="ExternalOutput")

    rg = [list(range(NCORES))]

    with tile.TileContext(nc) as tc:
        with tc.tile_pool(name="const", bufs=1) as cpool, \
             tc.tile_pool(name="main", bufs=1) as mpool, \
             tc.tile_pool(name="dram", bufs=1, space="DRAM") as dpool:
            ident = cpool.tile([P, P], FP, tag="ident")
            make_identity(nc, ident[:])
            ones1 = cpool.tile([1, P], FP, tag="ones1")
            nc.gpsimd.memset(ones1[:], 1.0)

            # long-lived per-head tensors
            qT = mpool.tile([D, TOK], FP, tag="qT")
            kT = mpool.tile([D, TOK], FP, tag="kT")
            vall = mpool.tile([P, TOK // P, DA], FP, tag="vall")
            gidxu = mpool.tile([P, NT], U32, tag="gidxu")
            msel = mpool.tile([P, NT], FP, tag="msel")

            # ================= phase A: DIFT NN map (ref-col sharded) ==========
            with nc.named_scope("phaseA"), \
                 tc.tile_pool(name="apool", bufs=1) as apool, \
                 tc.tile_pool(name="tns", bufs=3) as tns, \
                 tc.tile_pool(name="aps", bufs=1, space="PSUM") as aps:
                refn = apool.tile([P, CD], FP, tag="refn")
                nc.sync.dma_start(refn[:], refsh[:])
                sq = apool.tile([P, CD], FP, tag="sq")
                ssq = apool.tile([P, 1], FP, tag="ssq")
                nc.scalar.activation(sq[:], refn[:], AF.Square, accum_out=ssq[:])
                nrm = apool.tile([P, 1], FP, tag="nrm")
                nc.scalar.sqrt(nrm[:], ssq[:])
                nc.vector.tensor_scalar_add(nrm[:], nrm[:], 1e-8)
                sref = apool.tile([P, 1], FP, tag="sref")
                nc.vector.reciprocal(sref[:], nrm[:])
                rnn = apool.tile([P, CD], FP, tag="rnn")
                nc.scalar.activation(rnn[:], refn[:], AF.Copy, scale=sref[:])

                # transpose normalized ref rows -> rnT chunks [cd, ref]
                rnT = apool.tile([P, CDCH, P], FP, tag="rnT")
                for c_ in range(CDCH):
                    # share PSUM tags with the sim accumulators (used later)
                    pt = aps.tile([P, P], FP, tag=f"sim{c_ % 2}", name=f"ptr{c_}")
                    nc.tensor.transpose(pt[:], rnn[:, ts(c_, P)], ident[:])
                    nc.vector.tensor_copy(rnT[:, c_, :], pt[:])

                # tgt norm scale for this core's 128 tgt rows
                tgtn = apool.tile([P, CD], FP, tag="tgtn")
                nc.sync.dma_start(tgtn[:], tgtsh[:])
                sq2 = apool.tile([P, CD], FP, tag="sq2")
                ssq2 = apool.tile([P, 1], FP, tag="ssq2")
                nc.scalar.activation(sq2[:], tgtn[:], AF.Square, accum_out=ssq2[:])
                nrm2 = apool.tile([P, 1], FP, tag="nrm2")
                nc.scalar.sqrt(nrm2[:], ssq2[:])
                nc.vector.tensor_scalar_add(nrm2[:], nrm2[:], 1e-8)
                stgt = apool.tile([P, 1], FP, tag="stgt")
                nc.vector.reciprocal(stgt[:], nrm2[:])

                mv = apool.tile([1, SH], FP, tag="mv")
                nc.sync.dma_start(mv[:], maskv[:])

                lmax = apool.tile([P, NT, 8], FP, tag="lmax")
                lidx = apool.tile([P, NT, 8], U32, tag="lidx")
                # single pass over CD chunks, all 8 t-tiles accumulate in PSUM
                sims = [aps.tile([P, P], FP, tag=f"sim{i}", name=f"sim{i}")
                        for i in range(NT)]
                for c_ in range(CDCH):
                    tnt = tns.tile([P, S], FP, tag="tn")
                    nc.sync.dma_start(tnt[:], tnT[ts(c_, P), :])
                    for i in range(NT):
                        nc.tensor.matmul(
                            sims[i][:], lhsT=tnt[:, ts(i, P)], rhs=rnT[:, c_, :],
                            start=(c_ == 0), stop=False)
                for i in range(NT):
                    nc.tensor.matmul(sims[i][:], lhsT=ones1[:], rhs=mv[:],
                                     start=False, stop=True)
                for i in range(NT):
                    ssb = apool.tile([P, P], FP, tag=f"simsb{i % 2}")
                    nc.vector.tensor_copy(ssb[:], sims[i][:])
                    nc.vector.max(lmax[:, i, :], ssb[:])
                    nc.vector.max_index(lidx[:, i, :], lmax[:, i, :], ssb[:])

                ibt = apool.tile([P, 1], FP, tag="ibt")
                nc.sync.dma_start(ibt[:], ibase[:])
                lidxf = apool.tile([P, NT], FP, tag="lidxf")
                lmaxf = apool.tile([P, NT], FP, tag="lmaxf")
                nc.vector.tensor_copy(lidxf[:], lidx[:, :, 0])
                nc.vector.tensor_scalar_add(lidxf[:], lidxf[:], ibt[:, 0:1])
                nc.vector.tensor_copy(lmaxf[:], lmax[:, :, 0])

                agin = dpool.tile([P, 17], FP, tag="agin")
                agout = dpool.tile([P * NCORES, 17], FP, tag="agout",
                                   addr_space="Shared")
                nc.sync.dma_start(agin[:, 0:8], lmaxf[:])
                nc.sync.dma_start(agin[:, 8:16], lidxf[:])
                nc.sync.dma_start(agin[:, 16:17], stgt[:])
                nc.gpsimd.collective_compute(
                    "AllGather", OP.bypass,
                    ins=[agin[:].opt()], outs=[agout[:].opt()], replica_groups=rg)

                ag3 = agout[:].rearrange("(r p) f -> p r f", p=P)
                lmaxall = apool.tile([P, NCORES, NT], FP, tag="lmaxall")
                lidxall = apool.tile([P, NCORES, NT], FP, tag="lidxall")
                stgtf = apool.tile([P, NCORES], FP, tag="stgtf")
                nc.sync.dma_start(lmaxall[:], ag3[:, :, 0:8])
                nc.sync.dma_start(lidxall[:], ag3[:, :, 8:16])
                nc.sync.dma_start(stgtf[:], ag3[:, :, 16])

                gmax = apool.tile([P, NT], FP, tag="gmax")
                gidxf = apool.tile([P, NT], FP, tag="gidxf")
                nc.vector.tensor_copy(gmax[:], lmaxall[:, 0, :])
                nc.vector.tensor_copy(gidxf[:], lidxall[:, 0, :])
                gtt = apool.tile([P, NT], mybir.dt.uint8, tag="gtt")
                for r in range(1, NCORES):
                    nc.vector.tensor_tensor(gtt[:], lmaxall[:, r, :], gmax[:], op=OP.is_gt)
                    nc.vector.copy_predicated(gidxf[:], gtt[:], lidxall[:, r, :])
                    nc.vector.tensor_tensor(gmax[:], lmaxall[:, r, :], gmax[:], op=OP.max)

                dist = apool.tile([P, NT], FP, tag="dist")
                nc.vector.tensor_tensor(dist[:], gmax[:], stgtf[:], op=OP.mult)
                nc.vector.tensor_scalar(dist[:], dist[:], -1.0, 1.0, op0=OP.mult, op1=OP.add)
                nc.vector.tensor_scalar(msel[:], dist[:], THRESH, None, op0=OP.is_lt)
                nc.vector.tensor_copy(gidxu[:], gidxf[:])
                if debug_outputs:
                    nc.sync.dma_start(dbg_idx[:], gidxu[:])
                    nc.sync.dma_start(dbg_dist[:], dist[:])

            # ================= phase B: Q/K/V projections =====================
            with nc.named_scope("phaseB"), \
                 tc.tile_pool(name="xt", bufs=1) as xpool, \
                 tc.tile_pool(name="bps", bufs=3, space="PSUM") as bps:
                xts = xpool.tile([P, KCH, TOK], FP, tag="xt")
                for kc in range(KCH):
                    nc.sync.dma_start(xts[:, kc, :], x_T[ts(kc, P), :])
                wqt = xpool.tile([P, KCH, D], FP, tag="wqt")
                wkt = xpool.tile([P, KCH, D], FP, tag="wkt")
                wvt = xpool.tile([P, KCH, D], FP, tag="wvt")
                for wtile, wdram in ((wqt, wq_d), (wkt, wk_d), (wvt, wv_d)):
                    for kc in range(KCH):
                        nc.sync.dma_start(wtile[:, kc, :], wdram[ts(kc, P), :])

                for wtile, dst in ((wqt, qT), (wkt, kT)):
                    for n in range(TOK // 512):
                        psq = bps.tile([D, 512], FP, tag="proj")
                        for kc in range(KCH):
                            nc.tensor.matmul(
                                psq[:], lhsT=wtile[:, kc, :], rhs=xts[:, kc, ts(n, 512)],
                                start=(kc == 0), stop=(kc == KCH - 1))
                        if n % 2 == 0:
                            nc.scalar.copy(dst[:, ts(n, 512)], psq[:])
                        else:
                            nc.vector.tensor_copy(dst[:, ts(n, 512)], psq[:])

                for m in range(TOK // P):
                    psv = bps.tile([P, D], FP, tag="proj")
                    for kc in range(KCH):
                        nc.tensor.matmul(
                            psv[:], lhsT=xts[:, kc, ts(m, P)], rhs=wvt[:, kc, :],
                            start=(kc == 0), stop=(kc == KCH - 1))
                    nc.scalar.copy(vall[:, m, 0:D], psv[:])
                    nc.gpsimd.memset(vall[:, m, D:SUMROW], 0.0)
                    nc.gpsimd.memset(vall[:, m, SUMROW:DA], 1.0)

                # ============= phase C: NN K/V replacement for b=GEN ==========
                with nc.named_scope("phaseC"), \
                     tc.tile_pool(name="csb", bufs=2) as csb:
                    kref_d = dpool.tile([S, D], FP, tag="krefd")
                    vref_d = dpool.tile([S, D], FP, tag="vrefd")
                    for i in range(NT):
                        ptr = bps.tile([P, D], FP, tag="ctr")
                        nc.tensor.transpose(ptr[:], kT[:, ds(REF * S + i * P, P)],
                                            ident[0:D, 0:D])
                        krn = csb.tile([P, D], FP, tag="krn")
                        nc.vector.tensor_copy(krn[:], ptr[:])
                        nc.sync.dma_start(kref_d[ts(i, P), :], krn[:])
                        nc.sync.dma_start(vref_d[ts(i, P), :], vall[:, REF * NT + i, 0:D])

                    for i in range(NT):
                        krep = csb.tile([P, D], FP, tag="krep")
                        vrep = csb.tile([P, D], FP, tag="vrep")
                        nc.gpsimd.indirect_dma_start(
                            out=krep[:], out_offset=None, in_=kref_d[:],
                            in_offset=bass.IndirectOffsetOnAxis(ap=gidxu[:, i:i + 1], axis=0))
                        nc.gpsimd.indirect_dma_start(
                            out=vrep[:], out_offset=None, in_=vref_d[:],
                            in_offset=bass.IndirectOffsetOnAxis(ap=gidxu[:, i:i + 1], axis=0))
                        # k gen natural
                        ptg = bps.tile([P, D], FP, tag="ctr")
                        nc.tensor.transpose(ptg[:], kT[:, ds(GEN * S + i * P, P)],
                                            ident[0:D, 0:D])
                        kg = csb.tile([P, D], FP, tag="kg")
                        nc.vector.tensor_copy(kg[:], ptg[:])
                        kdiff = csb.tile([P, D], FP, tag="kdiff")
                        nc.vector.tensor_tensor(kdiff[:], krep[:], kg[:], op=OP.subtract)
                        knew = csb.tile([P, D], FP, tag="knew")
                        nc.vector.scalar_tensor_tensor(
                            knew[:], in0=kdiff[:], scalar=msel[:, i:i + 1], in1=kg[:],
                            op0=OP.mult, op1=OP.add)
                        ptb = bps.tile([D, P], FP, tag="ctr")
                        nc.tensor.transpose(ptb[:], knew[:], ident[:])
                        nc.vector.tensor_copy(kT[:, ds(GEN * S + i * P, P)], ptb[:])
                        # v blend (in place into vall)
                        vg = vall[:, GEN * NT + i, 0:D]
                        vdiff = csb.tile([P, D], FP, tag="vdiff")
                        nc.vector.tensor_tensor(vdiff[:], vrep[:], vg, op=OP.subtract)
                        nc.vector.scalar_tensor_tensor(
                            vg, in0=vdiff[:], scalar=msel[:, i:i + 1], in1=vg,
                            op0=OP.mult, op1=OP.add)

            # ================= phase D: attention per batch ===================
            outT_d = dpool.tile([D, TOK], FP, tag="outTd")
            outT_full = dpool.tile([C, TOK], FP, tag="outTfull",
                                   addr_space="Shared")
            with nc.named_scope("phaseD"), \
                 tc.tile_pool(name="scps", bufs=3, space="PSUM") as scps, \
                 tc.tile_pool(name="pvps", bufs=2, space="PSUM") as pvps, \
                 tc.tile_pool(name="prp", bufs=12) as prp, \
                 tc.tile_pool(name="dsb", bufs=3) as dsb:
                for b in range(B):
                    for icn in range(2):
                        prt = []
                        for jt in range(NT):
                            pss = scps.tile([P, 512], FP, tag="sc")
                            nc.tensor.matmul(
                                pss[:], lhsT=kT[:, ds(b * S + jt * P, P)],
                                rhs=qT[:, ds(b * S + icn * 512, 512)],
                                start=True, stop=True)
                            pet = prp.tile([P, 512], FP, tag="pr")
                            nc.scalar.activation(pet[:], pss[:], AF.Exp, scale=SCALE)
                            prt.append(pet)
                        po = pvps.tile([DA, 512], FP, tag="pv")
                        for jt in range(NT):
                            nc.tensor.matmul(
                                po[:], lhsT=vall[:, b * NT + jt, :], rhs=prt[jt][:],
                                start=(jt == 0), stop=(jt == NT - 1))
                        rc = dsb.tile([1, 512], FP, tag="rc")
                        nc.vector.reciprocal(rc[:], po[SUMROW:DA, :])
                        rb = dsb.tile([D, 512], FP, tag="rb")
                        nc.gpsimd.partition_broadcast(rb[:], rc[:])
                        ot = dsb.tile([D, 512], FP, tag="ot")
                        nc.vector.tensor_tensor(ot[:], po[0:D, :], rb[:], op=OP.mult)
                        nc.sync.dma_start(outT_d[:, ds(b * S + icn * 512, 512)], ot[:])

                nc.gpsimd.collective_compute(
                    "AllGather", OP.bypass,
                    ins=[outT_d[:].opt()], outs=[outT_full[:].opt()], replica_groups=rg)

            # ================= phase E: output projection (token-sharded) =====
            with nc.named_scope("phaseE"), \
                 tc.tile_pool(name="yps", bufs=2, space="PSUM") as yps, \
                 tc.tile_pool(name="ysb", bufs=1) as ysb:
                yid = ysb.tile([P, KCH], U32, tag="yid")
                nc.sync.dma_start(yid[:], yidx_d[:])
                otf_v = outT_full[:].rearrange("c (r n) -> (c r) n", n=NSL)
                xt_v = x_T[:].rearrange("c (r n) -> (c r) n", n=NSL)
                osl = ysb.tile([P, KCH, NSL], FP, tag="osl")
                xres = ysb.tile([P, KCH, NSL], FP, tag="xres")
                for m in range(KCH):
                    nc.gpsimd.indirect_dma_start(
                        out=osl[:, m, :], out_offset=None, in_=otf_v,
                        in_offset=bass.IndirectOffsetOnAxis(ap=yid[:, m:m + 1], axis=0))
                    nc.gpsimd.indirect_dma_start(
                        out=xres[:, m, :], out_offset=None, in_=xt_v,
                        in_offset=bass.IndirectOffsetOnAxis(ap=yid[:, m:m + 1], axis=0))
                wot = ysb.tile([P, KCH, C], FP, tag="wot")
                for kc in range(KCH):
                    nc.sync.dma_start(wot[:, kc, :], woT_d[ts(kc, P), :])
                bot = ysb.tile([P, KCH], FP, tag="bot")
                nc.sync.dma_start(bot[:], boc_d[:])
                for m in range(KCH):
                    yp = yps.tile([P, NSL], FP, tag="y")
                    for kc in range(KCH):
                        nc.tensor.matmul(
                            yp[:], lhsT=wot[:, kc, ts(m, P)], rhs=osl[:, kc, :],
                            start=(kc == 0), stop=(kc == KCH - 1))
                    yo = ysb.tile([P, NSL], FP, tag=f"yo{m % 2}")
                    nc.vector.scalar_tensor_tensor(
                        yo[:], in0=yp[:], scalar=bot[:, m:m + 1], in1=xres[:, m, :],
                        op0=OP.add, op1=OP.add)
                    nc.sync.dma_start(y_out[ts(m, P), :], yo[:])

            # ================= probe: float32r accuracy check =================
            if probe:
                with nc.named_scope("probe"), \
                     tc.tile_pool(name="pps", bufs=1, space="PSUM") as pps, \
                     tc.tile_pool(name="psb", bufs=1) as psb:
                    pat = psb.tile([P, P], FP, tag="pat")
                    pbt = psb.tile([P, 512], FP, tag="pbt")
                    nc.sync.dma_start(pat[:], pa_d[:])
                    nc.sync.dma_start(pbt[:], pb_d[:])
                    pp1 = pps.tile([P, 512], FP, tag="pp1")
                    nc.tensor.matmul(pp1[:], lhsT=pat[:], rhs=pbt[:], start=True, stop=True)
                    so1 = psb.tile([P, 512], FP, tag="so1")
                    nc.vector.tensor_copy(so1[:], pp1[:])
                    nc.sync.dma_start(probe32[:], so1[:])
                    part = psb.tile([P, P], FR, tag="part")
                    pbrt = psb.tile([P, 512], FR, tag="pbrt")
                    nc.sync.dma_start(part[:], par_d[:])
                    nc.sync.dma_start(pbrt[:], pbr_d[:])
                    pp2 = pps.tile([P, 512], FP, tag="pp2")
                    nc.tensor.matmul(pp2[:], lhsT=part[:], rhs=pbrt[:], start=True, stop=True)
                    so2 = psb.tile([P, 512], FP, tag="so2")
                    nc.vector.tensor_copy(so2[:], pp2[:])
                    nc.sync.dma_start(prober[:], so2[:])

    nc.compile()
    return nc


def _prep_inputs(inputs):
    hs = np.asarray(inputs["hidden_states"], dtype=np.float32)
    Wq = np.asarray(inputs["Wq"], dtype=np.float32)
    Wk = np.asarray(inputs["Wk"], dtype=np.float32)
    Wv = np.asarray(inputs["Wv"], dtype=np.float32)
    Wo = np.asarray(inputs["Wo"], dtype=np.float32)
    bo = np.asarray(inputs["bo"], dtype=np.float32)
    ref_dift = np.asarray(inputs["ref_dift"], dtype=np.float32)
    tgt_dift = np.asarray(inputs["tgt_dift"], dtype=np.float32)
    ref_mask = np.asarray(inputs["ref_mask"])

    x_T = np.ascontiguousarray(hs.reshape(TOK, C).T)
    tnT = np.ascontiguousarray(tgt_dift.T)
    WqT = np.ascontiguousarray(Wq.T)
    WkT = np.ascontiguousarray(Wk.T)
    WvT = np.ascontiguousarray(Wv.T)
    WoT = np.ascontiguousarray(Wo.T)
    bo_col = np.ascontiguousarray(bo.reshape(KCH, P).T)  # [128, 5]

    rng = np.random.default_rng(1234)
    pa = (rng.standard_normal((P, P)) * 0.1).astype(np.float32)
    pb = (rng.standard_normal((P, 512)) * 0.1).astype(np.float32)

    in_maps = []
    for r in range(NCORES):
        sl = slice(r * SH, (r + 1) * SH)
        hd = slice(r * D, (r + 1) * D)
        mvr = np.where(ref_mask[sl], 0.0, NEG).astype(np.float32).reshape(1, SH)
        cvals = np.arange(KCH) * P + np.arange(P)[:, None]       # [128, 5] global c
        yidx = (cvals * NCORES + r).astype(np.uint32)
        in_maps.append({
            "x_T": x_T,
            "tnT": tnT,
            "refsh": np.ascontiguousarray(ref_dift[sl]),
            "tgtsh": np.ascontiguousarray(tgt_dift[sl]),
            "maskv": mvr,
            "ibase": np.full((P, 1), r * SH, np.float32),
            "wq": np.ascontiguousarray(WqT[:, hd]),
            "wk": np.ascontiguousarray(WkT[:, hd]),
            "wv": np.ascontiguousarray(WvT[:, hd]),
            "woT": WoT,
            "boc": bo_col,
            "yidx": yidx,
            "pa": pa,
            "pb": pb,
            "par": pa,
            "pbr": pb,
        })
    return in_maps, (pa, pb)


_CACHED_NC = None


def kernel(**inputs):
    global LAST_RESULTS, _CACHED_NC
    debug = bool(int(os.environ.get("KERNEL_DEBUG", "0")))
    probe = bool(int(os.environ.get("KERNEL_PROBE", "0")))
    trace = bool(int(os.environ.get("KERNEL_TRACE", "0")))
    if _CACHED_NC is None:
        _CACHED_NC = build_program(debug_outputs=debug, probe=probe)
    nc = _CACHED_NC
    in_maps, _ = _prep_inputs(inputs)
    if not probe:
        for m in in_maps:
            m.pop("pa"), m.pop("pb"), m.pop("par"), m.pop("pbr")
    res = bass_utils.run_bass_kernel_spmd(
        nc, in_maps, core_ids=list(range(NCORES)), trace=trace)
    LAST_RESULTS = res
    yT = np.empty((C, TOK), np.float32)
    for r in range(NCORES):
        yT[:, r * NSL:(r + 1) * NSL] = res.results[r]["y_out"]
    out = np.ascontiguousarray(yT.T).reshape(B, S, C)
    return out


# revision 12
# speedup vs baseline: 1.8992x; 1.8992x over previous
"""Trainium2 Bass kernel for nn_AttnProcessor (DIFT nearest-neighbor sparse attention).

8-core SPMD: attention heads sharded across cores (1 head/core, all 4 batches);
the DIFT NN map is computed with ref-columns sharded (128 ref tokens/core) and
combined with a tiny AllGather; the output projection is token-sharded after an
AllGather of per-head attention outputs.

Precision: the NN similarity map runs in fp32 (argmax exactness: min top-2 gap
is ~1e-5); the attention/projection matmuls run in bf16 with fp32 PSUM
accumulation; the residual add is exact fp32.

Scheduling: attention for batches 0/1/3 is emitted before the NN K/V
replacement so it fills the NN-map AllGather latency; batch 2 (gen_cond) runs
after the replacement, which writes to separate tiles to avoid aliasing stalls.
"""
import os
import sys

for _p in ("/root/.axon_site/_ro/trn_rl_repo", "/opt/trn_rl_repo"):
    if os.path.isdir(_p) and _p not in sys.path:
        sys.path.append(_p)

import numpy as np

import concourse.bass as bass
import concourse.mybir as mybir
import concourse.tile as tile
from concourse import bacc
from concourse import bass_utils
from concourse.bass import ts, ds
from concourse.masks import make_identity

FP = mybir.dt.float32
BF = mybir.dt.bfloat16
U32 = mybir.dt.uint32
AF = mybir.ActivationFunctionType
OP = mybir.AluOpType

NCORES = 8
B, S, C, H, CD = 4, 1024, 640, 8, 1280
D = C // H              # 80 head dim
SUMROW = 96             # ones column lands on a valid partition base
DA = SUMROW + 1         # v augmented: cols [80,96) zero, col 96 = ones
TOK = B * S             # 4096
SH = S // NCORES        # 128 ref/tgt rows per core
P = 128
GEN, REF = 2, 3
SCALE = float(1.0 / np.sqrt(np.float32(D)))
NEG = -1e9
THRESH = 0.7
KCH = C // P            # 5 contraction chunks over C
CDCH = CD // P          # 10 contraction chunks over CD
NT = S // P             # 8 token tiles per batch
NSL = TOK // NCORES     # 512 output tokens per core

LAST_RESULTS = None


def build_program(debug_outputs=False):
    nc = bacc.Bacc("TRN2", target_bir_lowering=False, debug=False, num_devices=NCORES)

    x_T = nc.dram_tensor("x_T", [C, TOK], FP, kind="ExternalInput")
    x_Tb = nc.dram_tensor("x_Tb", [C, TOK], BF, kind="ExternalInput")
    tnT = nc.dram_tensor("tnT", [CD, S], FP, kind="ExternalInput")
    refsh = nc.dram_tensor("refsh", [SH, CD], FP, kind="ExternalInput")
    tgtsh = nc.dram_tensor("tgtsh", [SH, CD], FP, kind="ExternalInput")
    maskv = nc.dram_tensor("maskv", [1, SH], FP, kind="ExternalInput")
    ibase = nc.dram_tensor("ibase", [P, 1], FP, kind="ExternalInput")
    wq_d = nc.dram_tensor("wq", [C, D], BF, kind="ExternalInput")
    wk_d = nc.dram_tensor("wk", [C, D], BF, kind="ExternalInput")
    wv_d = nc.dram_tensor("wv", [C, D], BF, kind="ExternalInput")
    woT_d = nc.dram_tensor("woT", [C, C], BF, kind="ExternalInput")
    boc_d = nc.dram_tensor("boc", [P, KCH], FP, kind="ExternalInput")
    yidx_d = nc.dram_tensor("yidx", [P, KCH], U32, kind="ExternalInput")
    y_out = nc.dram_tensor("y_out", [C, NSL], FP, kind="ExternalOutput")
    if debug_outputs:
        dbg_idx = nc.dram_tensor("dbg_idx", [P, NT], U32, kind="ExternalOutput")
        dbg_dist = nc.dram_tensor("dbg_dist", [P, NT], FP, kind="ExternalOutput")

    rg = [list(range(NCORES))]

    with tile.TileContext(nc) as tc:
        with tc.tile_pool(name="const", bufs=1) as cpool, \
             tc.tile_pool(name="main", bufs=1) as mpool, \
             tc.tile_pool(name="dram", bufs=1, space="DRAM") as dpool:
            ident = cpool.tile([P, P], FP, tag="ident")
            make_identity(nc, ident[:])
            identr = cpool.tile([P, P], BF, tag="identr")
            nc.vector.tensor_copy(identr[:], ident[:])
            ones1 = cpool.tile([1, P], FP, tag="ones1")
            nc.gpsimd.memset(ones1[:], 1.0)

            # long-lived per-head tensors
            qT = mpool.tile([D, TOK], BF, tag="qT")
            kT = mpool.tile([D, TOK], BF, tag="kT")
            vT = mpool.tile([D, TOK], BF, tag="vT")
            vall = mpool.tile([P, TOK // P, DA], BF, tag="vall")
            # batch-2 replaced K/V live in separate tiles (no aliasing with b!=2 work)
            kTg = mpool.tile([D, S], BF, tag="kTg")
            vgn = mpool.tile([P, NT, DA], BF, tag="vgn")
            gidxu = mpool.tile([P, NT], U32, tag="gidxu")
            msel = mpool.tile([P, NT], FP, tag="msel")
            yid = mpool.tile([P, KCH], U32, tag="yid")
            nc.sync.dma_start(yid[:], yidx_d[:])

            # ================= phase A: DIFT NN map (ref-col sharded) ==========
            with nc.named_scope("phaseA"), \
                 tc.tile_pool(name="apool", bufs=1) as apool, \
                 tc.tile_pool(name="tns", bufs=3) as tns, \
                 tc.tile_pool(name="aps", bufs=1, space="PSUM") as aps:
                refn = apool.tile([P, CD], FP, tag="refn")
                nc.sync.dma_start(refn[:], refsh[:])
                sq = apool.tile([P, CD], FP, tag="sq")
                ssq = apool.tile([P, 1], FP, tag="ssq")
                nc.scalar.activation(sq[:], refn[:], AF.Square, accum_out=ssq[:])
                nrm = apool.tile([P, 1], FP, tag="nrm")
                nc.scalar.sqrt(nrm[:], ssq[:])
                nc.vector.tensor_scalar_add(nrm[:], nrm[:], 1e-8)
                sref = apool.tile([P, 1], FP, tag="sref")
                nc.vector.reciprocal(sref[:], nrm[:])
                rnn = apool.tile([P, CD], FP, tag="rnn")
                nc.scalar.activation(rnn[:], refn[:], AF.Copy, scale=sref[:])

                # transpose normalized ref rows -> rnT chunks [cd, ref]
                # (packed 4-per-PSUM-bank; two banks total, shared with sims)
                rnT = apool.tile([P, CDCH, P], FP, tag="rnT")
                ttr0 = aps.tile([P, 4, P], FP, tag="spack0", name="ttr0")
                ttr1 = aps.tile([P, 4, P], FP, tag="spack1", name="ttr1")
                for c_ in range(CDCH):
                    pt = (ttr0, ttr1)[c_ % 2][:, (c_ // 2) % 4, :]
                    nc.tensor.transpose(pt, rnn[:, ts(c_, P)], ident[:])
                    nc.vector.tensor_copy(rnT[:, c_, :], pt)

                # tgt norm scale for this core's 128 tgt rows
                tgtn = apool.tile([P, CD], FP, tag="tgtn")
                nc.sync.dma_start(tgtn[:], tgtsh[:])
                sq2 = apool.tile([P, CD], FP, tag="sq", name="sq2")
                ssq2 = apool.tile([P, 1], FP, tag="ssq2")
                nc.scalar.activation(sq2[:], tgtn[:], AF.Square, accum_out=ssq2[:])
                nrm2 = apool.tile([P, 1], FP, tag="nrm2")
                nc.scalar.sqrt(nrm2[:], ssq2[:])
                nc.vector.tensor_scalar_add(nrm2[:], nrm2[:], 1e-8)
                stgt = apool.tile([P, 1], FP, tag="stgt")
                nc.vector.reciprocal(stgt[:], nrm2[:])

                mv = apool.tile([1, SH], FP, tag="mv")
                nc.sync.dma_start(mv[:], maskv[:])

                lmax = apool.tile([P, NT, 8], FP, tag="lmax")
                lidx = apool.tile([P, NT, 8], U32, tag="lidx")
                # single pass over CD chunks; 8 t-tiles in 2 packed PSUM banks
                sp0 = aps.tile([P, 4, P], FP, tag="spack0", name="sp0")
                sp1 = aps.tile([P, 4, P], FP, tag="spack1", name="sp1")
                for c_ in range(CDCH):
                    tnt = tns.tile([P, S], FP, tag="tn")
                    nc.sync.dma_start(tnt[:], tnT[ts(c_, P), :])
                    for i in range(NT):
                        spt = (sp0, sp1)[i // 4][:, i % 4, :]
                        nc.tensor.matmul(
                            spt, lhsT=tnt[:, ts(i, P)], rhs=rnT[:, c_, :],
                            start=(c_ == 0), stop=False)
                for i in range(NT):
                    spt = (sp0, sp1)[i // 4][:, i % 4, :]
                    nc.tensor.matmul(spt, lhsT=ones1[:], rhs=mv[:],
                                     start=False, stop=True)
                for i in range(NT):
                    spt = (sp0, sp1)[i // 4][:, i % 4, :]
                    ssb = apool.tile([P, P], FP, tag=f"simsb{i % 2}")
                    nc.vector.tensor_copy(ssb[:], spt)
                    nc.vector.max(lmax[:, i, :], ssb[:])
                    nc.vector.max_index(lidx[:, i, :], lmax[:, i, :], ssb[:])

                ibt = apool.tile([P, 1], FP, tag="ibt")
                nc.sync.dma_start(ibt[:], ibase[:])
                lidxf = apool.tile([P, NT], FP, tag="lidxf")
                lmaxf = apool.tile([P, NT], FP, tag="lmaxf")
                nc.vector.tensor_copy(lidxf[:], lidx[:, :, 0])
                nc.vector.tensor_scalar_add(lidxf[:], lidxf[:], ibt[:, 0:1])
                nc.vector.tensor_copy(lmaxf[:], lmax[:, :, 0])

                agin = dpool.tile([P, 17], FP, tag="agin")
                agout = dpool.tile([P * NCORES, 17], FP, tag="agout",
                                   addr_space="Shared")
                nc.sync.dma_start(agin[:, 0:8], lmaxf[:])
                nc.sync.dma_start(agin[:, 8:16], lidxf[:])
                nc.sync.dma_start(agin[:, 16:17], stgt[:])
                nc.gpsimd.collective_compute(
                    "AllGather", OP.bypass,
                    ins=[agin[:].opt()], outs=[agout[:].opt()], replica_groups=rg)

                ag3 = agout[:].rearrange("(r p) f -> p r f", p=P)
                lmaxall = apool.tile([P, NCORES, NT], FP, tag="lmaxall")
                lidxall = apool.tile([P, NCORES, NT], FP, tag="lidxall")
                stgtf = apool.tile([P, NCORES], FP, tag="stgtf")
                nc.sync.dma_start(lmaxall[:], ag3[:, :, 0:8])
                nc.sync.dma_start(lidxall[:], ag3[:, :, 8:16])
                nc.sync.dma_start(stgtf[:], ag3[:, :, 16])

                gmax = apool.tile([P, NT], FP, tag="gmax")
                gidxf = apool.tile([P, NT], FP, tag="gidxf")
                nc.vector.tensor_copy(gmax[:], lmaxall[:, 0, :])
                nc.vector.tensor_copy(gidxf[:], lidxall[:, 0, :])
                gtt = apool.tile([P, NT], mybir.dt.uint8, tag="gtt")
                for r in range(1, NCORES):
                    nc.vector.tensor_tensor(gtt[:], lmaxall[:, r, :], gmax[:], op=OP.is_gt)
                    nc.vector.copy_predicated(gidxf[:], gtt[:], lidxall[:, r, :])
                    nc.vector.tensor_tensor(gmax[:], lmaxall[:, r, :], gmax[:], op=OP.max)

                dist = apool.tile([P, NT], FP, tag="dist")
                nc.vector.tensor_tensor(dist[:], gmax[:], stgtf[:], op=OP.mult)
                nc.vector.tensor_scalar(dist[:], dist[:], -1.0, 1.0, op0=OP.mult, op1=OP.add)
                nc.vector.tensor_scalar(msel[:], dist[:], THRESH, None, op0=OP.is_lt)
                nc.vector.tensor_copy(gidxu[:], gidxf[:])
                if debug_outputs:
                    nc.sync.dma_start(dbg_idx[:], gidxu[:])
                    nc.sync.dma_start(dbg_dist[:], dist[:])

            # ctr PSUM pool spans B..D (transposes); 2 banks
            with tc.tile_pool(name="ctrp", bufs=2, space="PSUM") as ctrp:
                # ================= phase B: Q/K/V projections =================
                with nc.named_scope("phaseB"), \
                     tc.tile_pool(name="xt", bufs=1) as xpool, \
                     tc.tile_pool(name="bps", bufs=3, space="PSUM") as bps:
                    xts = xpool.tile([P, KCH, TOK], BF, tag="xt")
                    for kc in range(KCH):
                        nc.scalar.dma_start(xts[:, kc, :], x_Tb[ts(kc, P), :])
                    wqt = xpool.tile([P, KCH, D], BF, tag="wqt")
                    wkt = xpool.tile([P, KCH, D], BF, tag="wkt")
                    wvt = xpool.tile([P, KCH, D], BF, tag="wvt")
                    for wtile, wdram in ((wqt, wq_d), (wkt, wk_d), (wvt, wv_d)):
                        for kc in range(KCH):
                            nc.scalar.dma_start(wtile[:, kc, :], wdram[ts(kc, P), :])

                    for wtile, dst in ((wkt, kT), (wqt, qT), (wvt, vT)):
                        for n in range(TOK // 512):
                            psq = bps.tile([D, 512], FP, tag="proj")
                            for kc in range(KCH):
                                nc.tensor.matmul(
                                    psq[:], lhsT=wtile[:, kc, :], rhs=xts[:, kc, ts(n, 512)],
                                    start=(kc == 0), stop=(kc == KCH - 1))
                            if n % 2 == 0:
                                nc.scalar.copy(dst[:, ts(n, 512)], psq[:])
                            else:
                                nc.vector.tensor_copy(dst[:, ts(n, 512)], psq[:])

                    # v pad columns: zeros + the ones column
                    nc.gpsimd.memset(vall[:, :, D:SUMROW], 0.0)
                    nc.gpsimd.memset(vall[:, :, SUMROW:DA], 1.0)
                    nc.gpsimd.memset(vgn[:, :, D:SUMROW], 0.0)
                    nc.gpsimd.memset(vgn[:, :, SUMROW:DA], 1.0)
                    # v natural tiles via PE transpose of vT
                    for m in range(TOK // P):
                        psv = ctrp.tile([P, D], BF, tag="ctr", name=f"psv{m}")
                        nc.tensor.transpose(psv[:], vT[:, ts(m, P)], identr[0:D, 0:D])
                        if m % 2 == 0:
                            nc.scalar.copy(vall[:, m, 0:D], psv[:])
                        else:
                            nc.vector.tensor_copy(vall[:, m, 0:D], psv[:])

                    # stage ref-batch K/V to DRAM for the NN gather (needs only B)
                    kref_d = dpool.tile([S, D], BF, tag="krefd")
                    vref_d = dpool.tile([S, D], BF, tag="vrefd")
                    with tc.tile_pool(name="csb0", bufs=2) as csb0:
                        for i in range(NT):
                            ptr = ctrp.tile([P, D], BF, tag="ctr", name=f"ptc{i}")
                            nc.tensor.transpose(ptr[:], kT[:, ds(REF * S + i * P, P)],
                                                identr[0:D, 0:D])
                            krn = csb0.tile([P, D], BF, tag="krn")
                            nc.vector.tensor_copy(krn[:], ptr[:])
                            nc.sync.dma_start(kref_d[ts(i, P), :], krn[:])
                            nc.sync.dma_start(vref_d[ts(i, P), :], vall[:, REF * NT + i, 0:D])

                # ================= phase D + C interleaved ====================
                outT_d = dpool.tile([D, TOK], BF, tag="outTd")
                outT_full = dpool.tile([C, TOK], BF, tag="outTfull",
                                       addr_space="Shared")
                with nc.named_scope("phaseD"), \
                     tc.tile_pool(name="scps", bufs=3, space="PSUM") as scps, \
                     tc.tile_pool(name="pvps", bufs=2, space="PSUM") as pvps, \
                     tc.tile_pool(name="prp", bufs=12) as prp, \
                     tc.tile_pool(name="dsb", bufs=3) as dsb, \
                     tc.tile_pool(name="csb", bufs=2) as csb:

                    def attn_batch(b, kT_b, v_b):
                        for icn in range(2):
                            prt = []
                            for jt in range(NT):
                                pss = scps.tile([P, 512], FP, tag="sc",
                                                name=f"pss{b}_{icn}_{jt}")
                                nc.tensor.matmul(
                                    pss[:], lhsT=kT_b[:, ts(jt, P)],
                                    rhs=qT[:, ds(b * S + icn * 512, 512)],
                                    start=True, stop=True)
                                pet = prp.tile([P, 512], BF, tag="pr",
                                               name=f"pet{b}_{icn}_{jt}")
                                nc.scalar.activation(pet[:], pss[:], AF.Exp, scale=SCALE)
                                prt.append(pet)
                            po = pvps.tile([DA, 512], FP, tag="pv",
                                           name=f"po{b}_{icn}")
                            for jt in range(NT):
                                nc.tensor.matmul(
                                    po[:], lhsT=v_b[:, jt, :], rhs=prt[jt][:],
                                    start=(jt == 0), stop=(jt == NT - 1))
                            rc = dsb.tile([1, 512], FP, tag="rc", name=f"rc{b}_{icn}")
                            nc.vector.reciprocal(rc[:], po[SUMROW:DA, :])
                            rb = dsb.tile([D, 512], FP, tag="rb", name=f"rb{b}_{icn}")
                            nc.gpsimd.partition_broadcast(rb[:], rc[:])
                            ot = dsb.tile([D, 512], BF, tag="ot", name=f"ot{b}_{icn}")
                            nc.vector.tensor_tensor(ot[:], po[0:D, :], rb[:], op=OP.mult)
                            nc.sync.dma_start(outT_d[:, ds(b * S + icn * 512, 512)], ot[:])

                    # batches that do not depend on the NN map run first,
                    # hiding the AllGather latency
                    for b in (0, 1, 3):
                        attn_batch(b, kT[:, ds(b * S, S)],
                                   vall[:, b * NT:(b + 1) * NT, :])

                    # ---- phase C: build replaced K/V for b=GEN ----
                    with nc.named_scope("phaseC"):
                        for i in range(NT):
                            krep = csb.tile([P, D], BF, tag="krep")
                            vrep = csb.tile([P, D], BF, tag="vrep")
                            nc.gpsimd.indirect_dma_start(
                                out=krep[:], out_offset=None, in_=kref_d[:],
                                in_offset=bass.IndirectOffsetOnAxis(
                                    ap=gidxu[:, i:i + 1], axis=0))
                            nc.gpsimd.indirect_dma_start(
                                out=vrep[:], out_offset=None, in_=vref_d[:],
                                in_offset=bass.IndirectOffsetOnAxis(
                                    ap=gidxu[:, i:i + 1], axis=0))
                            # k gen natural
                            ptg = ctrp.tile([P, D], BF, tag="ctr", name=f"ptg{i}")
                            nc.tensor.transpose(ptg[:], kT[:, ds(GEN * S + i * P, P)],
                                                identr[0:D, 0:D])
                            kg = csb.tile([P, D], BF, tag="kg")
                            nc.vector.tensor_copy(kg[:], ptg[:])
                            kdiff = csb.tile([P, D], BF, tag="kdiff")
                            nc.vector.tensor_tensor(kdiff[:], krep[:], kg[:], op=OP.subtract)
                            knew = csb.tile([P, D], BF, tag="knew")
                            nc.vector.scalar_tensor_tensor(
                                knew[:], in0=kdiff[:], scalar=msel[:, i:i + 1], in1=kg[:],
                                op0=OP.mult, op1=OP.add)
                            ptb = ctrp.tile([D, P], BF, tag="ctr", name=f"ptb{i}")
                            nc.tensor.transpose(ptb[:], knew[:], identr[:])
                            nc.vector.tensor_copy(kTg[:, ts(i, P)], ptb[:])
                            # v blend into the separate vgn tile
                            vg = vall[:, GEN * NT + i, 0:D]
                            vdiff = csb.tile([P, D], BF, tag="vdiff")
                            nc.vector.tensor_tensor(vdiff[:], vrep[:], vg, op=OP.subtract)
                            nc.vector.scalar_tensor_tensor(
                                vgn[:, i, 0:D], in0=vdiff[:], scalar=msel[:, i:i + 1],
                                in1=vg, op0=OP.mult, op1=OP.add)

                    # gen batch with replaced K/V
                    attn_batch(GEN, kTg, vgn)

                    nc.gpsimd.collective_compute(
                        "AllGather", OP.bypass,
                        ins=[outT_d[:].opt()], outs=[outT_full[:].opt()],
                        replica_groups=rg)

            # ================= phase E: output projection (token-sharded) =====
            with nc.named_scope("phaseE"), \
                 tc.tile_pool(name="yps", bufs=2, space="PSUM") as yps, \
                 tc.tile_pool(name="ysb", bufs=1) as ysb:
                xres = ysb.tile([P, KCH, NSL], FP, tag="xres")
                xt_v = x_T[:].rearrange("c (r n) -> (c r) n", n=NSL)
                for m in range(KCH):
                    nc.gpsimd.indirect_dma_start(
                        out=xres[:, m, :], out_offset=None, in_=xt_v,
                        in_offset=bass.IndirectOffsetOnAxis(ap=yid[:, m:m + 1], axis=0))
                wot = ysb.tile([P, KCH, C], BF, tag="wot")
                for kc in range(KCH):
                    nc.scalar.dma_start(wot[:, kc, :], woT_d[ts(kc, P), :])
                bot = ysb.tile([P, KCH], FP, tag="bot")
                nc.sync.dma_start(bot[:], boc_d[:])
                otf_v = outT_full[:].rearrange("c (r n) -> (c r) n", n=NSL)
                osl = ysb.tile([P, KCH, NSL], BF, tag="osl")
                for m in range(KCH):
                    nc.gpsimd.indirect_dma_start(
                        out=osl[:, m, :], out_offset=None, in_=otf_v,
                        in_offset=bass.IndirectOffsetOnAxis(ap=yid[:, m:m + 1], axis=0))
                for m in range(KCH):
                    yp = yps.tile([P, NSL], FP, tag="y")
                    for kc in range(KCH):
                        nc.tensor.matmul(
                            yp[:], lhsT=wot[:, kc, ts(m, P)], rhs=osl[:, kc, :],
                            start=(kc == 0), stop=(kc == KCH - 1))
                    yo = ysb.tile([P, NSL], FP, tag=f"yo{m % 2}")
                    nc.vector.scalar_tensor_tensor(
                        yo[:], in0=yp[:], scalar=bot[:, m:m + 1], in1=xres[:, m, :],
                        op0=OP.add, op1=OP.add)
                    nc.sync.dma_start(y_out[ts(m, P), :], yo[:])

    nc.compile()
    return nc


def _prep_inputs(inputs):
    import ml_dtypes
    hs = np.asarray(inputs["hidden_states"], dtype=np.float32)
    Wq = np.asarray(inputs["Wq"], dtype=np.float32)
    Wk = np.asarray(inputs["Wk"], dtype=np.float32)
    Wv = np.asarray(inputs["Wv"], dtype=np.float32)
    Wo = np.asarray(inputs["Wo"], dtype=np.float32)
    bo = np.asarray(inputs["bo"], dtype=np.float32)
    ref_dift = np.asarray(inputs["ref_dift"], dtype=np.float32)
    tgt_dift = np.asarray(inputs["tgt_dift"], dtype=np.float32)
    ref_mask = np.asarray(inputs["ref_mask"])

    x_T = np.ascontiguousarray(hs.reshape(TOK, C).T)
    x_Tb = x_T.astype(ml_dtypes.bfloat16)
    tnT = np.ascontiguousarray(tgt_dift.T)
    WqT = np.ascontiguousarray(Wq.T)
    WkT = np.ascontiguousarray(Wk.T)
    WvT = np.ascontiguousarray(Wv.T)
    WoT = np.ascontiguousarray(Wo.T).astype(ml_dtypes.bfloat16)
    bo_col = np.ascontiguousarray(bo.reshape(KCH, P).T)  # [128, 5]

    in_maps = []
    for r in range(NCORES):
        sl = slice(r * SH, (r + 1) * SH)
        hd = slice(r * D, (r + 1) * D)
        mvr = np.where(ref_mask[sl], 0.0, NEG).astype(np.float32).reshape(1, SH)
        cvals = np.arange(KCH) * P + np.arange(P)[:, None]       # [128, 5] global c
        yidx = (cvals * NCORES + r).astype(np.uint32)
        in_maps.append({
            "x_T": x_T,
            "x_Tb": x_Tb,
            "tnT": tnT,
            "refsh": np.ascontiguousarray(ref_dift[sl]),
            "tgtsh": np.ascontiguousarray(tgt_dift[sl]),
            "maskv": mvr,
            "ibase": np.full((P, 1), r * SH, np.float32),
            "wq": np.ascontiguousarray(WqT[:, hd]).astype(ml_dtypes.bfloat16),
            "wk": np.ascontiguousarray(WkT[:, hd]).astype(ml_dtypes.bfloat16),
            "wv": np.ascontiguousarray(WvT[:, hd]).astype(ml_dtypes.bfloat16),
            "woT": WoT,
            "boc": bo_col,
            "yidx": yidx,
        })
    return in_maps, None


_CACHED_NC = None


def kernel(**inputs):
    global LAST_RESULTS, _CACHED_NC
    debug = bool(int(os.environ.get("KERNEL_DEBUG", "0")))
    trace = bool(int(os.environ.get("KERNEL_TRACE", "0")))
    if _CACHED_NC is None:
        _CACHED_NC = build_program(debug_outputs=debug)
    nc = _CACHED_NC
    in_maps, _ = _prep_inputs(inputs)
    res = bass_utils.run_bass_kernel_spmd(
        nc, in_maps, core_ids=list(range(NCORES)), trace=trace)
    LAST_RESULTS = res
    yT = np.empty((C, TOK), np.float32)
    for r in range(NCORES):
        yT[:, r * NSL:(r + 1) * NSL] = res.results[r]["y_out"]
    out = np.ascontiguousarray(yT.T).reshape(B, S, C)
    return out
